# revision 59
# baseline (speedup 1.0000x reference)
"""Causal multi-head attention block on 8 Trainium2 NeuronCores.

Problem: x[4,2048,1024] -> QKV proj (16 heads, dh=64) -> causal softmax
attention -> out proj. Sharding: core = (batch, head-half): each core
computes QKV for 8 heads of one batch, flash-style attention for those
heads, and a partial O-projection over its 512 W_o input columns; the
host sums the two partials per batch (tensor-parallel unshard).

Device kernel (identical SPMD program, per-core data), all matmuls bf16
with fp32 PSUM accumulation:
  - x.T host pre-transposed; Q.T/K.T computed in [o, t] feature-major
    layout, V in [t, o]. K carries no bias (softmax over k is invariant
    to per-q constants, so only Q needs b_q); the V bias is folded into
    the output bias on the host (attn rows sum to 1).
  - scores are computed transposed, S.T[k_tile, q_span] = K.T_blk^T@Q.T,
    two k-tiles side by side in one 2-bank PSUM tile so ScalarE exps
    them in one ACTIVATE (scale=1/8 folded in; scores are O(1) so no
    max-subtraction). Diagonal blocks are masked after exp with a 0/1
    triangle multiply on GpSimd (keeps DVE off the exp->PV chain).
  - O.T[c, q] accumulates with V' stationary: V' = [ones(64) | V(64)]
    for every head, so the matmul broadcasts the softmax denominator
    into rows 0:64 and O.T lands on rows 64:128. Normalization is one
    reciprocal_approx_fast on the PSUM denominator rows (base 0, no
    copy) + one cross-partition-base multiply into OT[c, t].
  - the attention work is one flat pair stream ordered J-outer (phase J
    = all 8 heads' J-th block-row), with a depth-2 score-PSUM pipeline
    and the P@V matmuls lagging 2-3 pairs behind the exps; the lag
    crosses head/phase boundaries so the exp->mask->P@V chain never
    drains mid-kernel. J-outer also spreads the natural PE filler
    (V tiles per phase, next phase's Q/K projection units, O-projection
    of the previous phase) across the whole run, so the stretches where
    ScalarE's exp throughput exceeds PE's attention work always have
    independent projection matmuls to chew on. Projection units run
    from a dedicated PSUM pool so they are never gated on exp draining
    a score tile; input DMAs are chunked and ordered so each unit's
    data lands just ahead of it, and partial outputs stream back in
    bf16 on alternating DMA queues (the host sums the two per-batch
    partials in fp32).
"""

import numpy as np
import ml_dtypes

BF16 = ml_dtypes.bfloat16

B, T, D = 4, 2048, 1024
NH, DH = 16, 64
HPC = 8            # heads per core
OC = HPC * DH      # 512: per-core head columns
NT = T // 128      # 16 q/k tiles of 128
ND = D // 128      # 8 d-tiles
N_CORES = 8

_cache = {}


def _build(debug=False):
    import concourse.mybir as mybir
    import concourse.tile as tile
    from concourse import bacc

    f32 = mybir.dt.float32
    bf16 = mybir.dt.bfloat16
    Exp = mybir.ActivationFunctionType.Exp
    Mult = mybir.AluOpType.mult
    Add = mybir.AluOpType.add

    nc = bacc.Bacc("TRN2", target_bir_lowering=False, debug=False,
                   num_devices=N_CORES)

    xT = nc.declare_dram_parameter("xT", [D, T], bf16, isOutput=False)
    wqk = nc.declare_dram_parameter("wqkT", [D, 2 * OC], bf16, isOutput=False)
    wv = nc.declare_dram_parameter("wvT", [D, OC], bf16, isOutput=False)
    wo = nc.declare_dram_parameter("woT", [OC, D], bf16, isOutput=False)
    bq = nc.declare_dram_parameter("bq", [128, OC // 128], f32, isOutput=False)
    bo = nc.declare_dram_parameter("bo", [1, D], f32, isOutput=False)
    tri = nc.declare_dram_parameter("tri", [128, 128], bf16, isOutput=False)
    out = nc.declare_dram_parameter("out", [T, D], bf16, isOutput=True)
    if debug:
        d_qkt = nc.declare_dram_parameter("d_qkt", [128, ND * T], bf16, isOutput=True)
        d_vp = nc.declare_dram_parameter(
            "d_vp", [128, NT * HPC * 128], bf16, isOutput=True)
        d_ot = nc.declare_dram_parameter(
            "d_ot", [128, (OC // 128) * T], bf16, isOutput=True)

    with tile.TileContext(nc) as tc:
        with (
            tc.tile_pool(name="persist", bufs=1) as persist,
            tc.tile_pool(name="pt", bufs=6) as ptp,
            tc.tile_pool(name="dn", bufs=3) as dnp,
            tc.tile_pool(name="ostage", bufs=4) as ostage,
            tc.tile_pool(name="psS", bufs=2, space="PSUM") as psS,
            tc.tile_pool(name="psP", bufs=2, space="PSUM") as psP,
            tc.tile_pool(name="psO", bufs=2, space="PSUM") as psO,
        ):
            # ---- persistent SBUF tensors ----
            XT = persist.tile([128, ND, T], bf16)          # x.T d-tiles
            WQK = persist.tile([128, ND, 2 * OC], bf16)
            WV = persist.tile([128, ND, OC], bf16)
            WO = persist.tile([128, OC // 128, D], bf16)
            BQ = persist.tile([128, OC // 128], f32)
            BO = persist.tile([128, D], f32)
            TRI = persist.tile([128, 128], bf16)
            QKT = persist.tile([128, ND, T], bf16)         # [o, t] Q.T|K.T
            # V' per head, 128 cols: [1*64 | V(64)]. The P@V matmul lands the
            # softmax denominator on otr rows 0:64 and O.T on rows 64:128.
            VP = persist.tile([128, NT, HPC, 128], bf16)
            OT = persist.tile([128, OC // 128, T], bf16)   # attn out.T [c, t]

            # warm-up: keep PE busy (and the clock ramped) while the input
            # DMAs stream in; results are never read.
            JNK = persist.tile([128, 512], bf16)
            nc.vector.memset(JNK[:, 0:128], 0.5)
            jp0 = psP.tile([128, 512], f32, tag="p", name="jnk0")
            for m in range(4):
                nc.tensor.matmul(
                    jp0[:, 0:128], lhsT=JNK[:, 0:128], rhs=JNK[:, 0:128],
                    start=(m == 0), stop=(m == 3),
                )
            nc.vector.memset(JNK[:, 128:512], 0.5)
            jp1 = psP.tile([128, 512], f32, tag="p", name="jnk1")
            for m in range(9):
                nc.tensor.matmul(
                    jp1[:], lhsT=JNK[:, 0:128], rhs=JNK[:],
                    start=(m == 0), stop=(m == 8),
                )

            # ---- input DMAs: coarse chunks, priority order ----
            # Issue rate is the startup constraint (~0.6-1us per dma_start
            # per queue), so the critical first 2.25MB goes as 4 big chunks
            # staggered across the three queues.
            xTr = xT.rearrange("(n p) t -> p n t", p=128)
            wqkr = wqk.rearrange("(n p) o -> p n o", p=128)
            wvr = wv.rearrange("(n p) o -> p n o", p=128)
            wor = wo.rearrange("(n p) o -> p n o", p=128)

            def X(q, kds, tch):
                q.dma_start(
                    out=XT[:, kds, tch * 512:(tch + 1) * 512],
                    in_=xTr[:, kds, tch * 512:(tch + 1) * 512])

            def WQ(q, pair):       # 256-col block (2 ot), all kd
                q.dma_start(
                    out=WQK[:, :, pair * 256:(pair + 1) * 256],
                    in_=wqkr[:, :, pair * 256:(pair + 1) * 256])

            lo, hi = slice(0, 4), slice(4, 8)
            # critical: x(tch0) + Q/K weights for heads 0-3. x goes in 2-kd
            # slices alternating queues so the first QKV unit's kd matmuls
            # start as the chunks land instead of after one 1MB transfer.
            WQ(nc.gpsimd, 0)       # ot 0,1 -- first on its queue
            X(nc.sync, slice(0, 2), 0)
            X(nc.scalar, slice(2, 4), 0)
            X(nc.sync, slice(4, 6), 0)
            X(nc.scalar, slice(6, 8), 0)
            WQ(nc.sync, 2)         # ot 4,5
            nc.scalar.dma_start(out=TRI[:], in_=tri[:, :])
            nc.scalar.dma_start(out=BQ[:], in_=bq[:, :])
            # V' ones columns (PV of h0/J0 needs the first tiles)
            nc.gpsimd.memset(VP[:, 0:4, :, 0:DH], 1.0)
            # x span 1, then wv: the first v unit isn't consumed until the
            # J0 attention is underway, so wv must not crowd the critical x
            X(nc.gpsimd, lo, 1)
            X(nc.sync, hi, 1)
            nc.scalar.dma_start(out=WV[:, lo, :], in_=wvr[:, lo, :])
            nc.sync.dma_start(out=WV[:, hi, :], in_=wvr[:, hi, :])
            nc.scalar.dma_start(out=BO[:], in_=bo[:, :].to_broadcast((128, D)))
            nc.gpsimd.memset(VP[:, 4:NT, :, 0:DH], 1.0)
            X(nc.sync, lo, 2)
            X(nc.gpsimd, hi, 2)
            X(nc.sync, lo, 3)
            X(nc.gpsimd, hi, 3)
            # Q/K weights for heads 4-7 (consumed from h2 on), W_o (h7)
            WQ(nc.sync, 1)         # ot 2,3
            WQ(nc.gpsimd, 3)       # ot 6,7
            nc.sync.dma_start(out=WO[:, 0:2, :], in_=wor[:, 0:2, :])
            nc.gpsimd.dma_start(out=WO[:, 2:4, :], in_=wor[:, 2:4, :])

            # ---- projection units (dedicated psP pool) ----
            def emit_qk(ot, tch):
                # one [o, t] chunk: [128 o, 512 t] = W_qk @ x.T (+ b_q)
                ps = psP.tile([128, 512], f32, tag="p", name=f"qk{ot}_{tch}")
                for kd in range(ND):
                    nc.tensor.matmul(
                        ps[:],
                        lhsT=WQK[:, kd, ot * 128:(ot + 1) * 128],
                        rhs=XT[:, kd, tch * 512:(tch + 1) * 512],
                        start=(kd == 0), stop=(kd == ND - 1),
                    )
                dst = QKT[:, ot, tch * 512:(tch + 1) * 512]
                if ot < 4:
                    nc.vector.tensor_scalar_add(dst, ps[:], BQ[:, ot:ot + 1])
                else:
                    nc.vector.tensor_copy(dst, ps[:])

            def emit_v(tt):
                # one [t, o] tile of V = x @ W_v.T into V' cols 64:128
                ps = psP.tile([128, 512], f32, tag="p", name=f"v{tt}")
                for kd in range(ND):
                    nc.tensor.matmul(
                        ps[:],
                        lhsT=XT[:, kd, tt * 128:(tt + 1) * 128],
                        rhs=WV[:, kd, :],
                        start=(kd == 0), stop=(kd == ND - 1),
                    )
                nc.vector.tensor_copy(
                    VP[:, tt, :, DH:128],
                    ps[:].rearrange("p (a b) -> p a b", b=DH),
                )

            def emit_oproj(tq, oc2, split=False):
                # out[tq, oc2] = O @ WoT + bo' (partial over this core's
                # 512 W_o input columns; bo' host-folded with the V bias).
                # split: halve the free dim so the bias/DMA of the first
                # half overlaps the second half's matmuls (tail latency).
                ps = psP.tile([128, 512], f32, tag="p", name=f"op{tq}_{oc2}")
                # alternate output queues so the drain isn't serialized; the
                # tail (tq>=12) splits between sync and the then-idle scalar
                if tq >= 12:
                    outq = nc.sync if (tq + oc2) % 2 == 0 else nc.scalar
                else:
                    outq = nc.sync if (tq + oc2) % 2 == 0 else nc.gpsimd
                nh = 2 if split else 1
                w = 512 // nh
                for half in range(nh):
                    c0 = half * w
                    for ct in range(OC // 128):
                        nc.tensor.matmul(
                            ps[:, c0:c0 + w],
                            lhsT=OT[:, ct, tq * 128:(tq + 1) * 128],
                            rhs=WO[:, ct, oc2 * 512 + c0:oc2 * 512 + c0 + w],
                            start=(ct == 0), stop=(ct == OC // 128 - 1),
                        )
                    ob = ostage.tile([128, 512], bf16, tag="ob")
                    nc.vector.tensor_tensor(
                        out=ob[:, 0:w], in0=ps[:, c0:c0 + w],
                        in1=BO[:, oc2 * 512 + c0:oc2 * 512 + c0 + w], op=Add,
                    )
                    outq.dma_start(
                        out=out[tq * 128:(tq + 1) * 128,
                                oc2 * 512 + c0:oc2 * 512 + c0 + w],
                        in_=ob[:, 0:w],
                    )

            # ---- fill schedule: slot -> units ----
            # J-outer stream: phase J = all 8 heads' J-th block-row.
            # Phase starts: J0@0 (2 iters/head), J1@16 (4), J2@48 (6),
            # J3@96 (8). Head h's phase-J iters start at phase + h*(2J+2).
            # Every unit lands a few slots before its first consumer, and
            # fill is spread so each ACT-heavy stretch keeps PE fed.
            sched = {}

            def put(slot, *unit):
                sched.setdefault(slot, []).append(unit)

            # v tiles: tt consumed from phase tt//4 by head 0
            for s, tt in zip((0, 0, 1, 1), range(4)):
                put(s, "v", tt)
            for s, tt in zip((16, 17, 18, 19), range(4, 8)):
                put(s, "v", tt)
            for s, tt in zip((48, 49, 50, 51), range(8, 12)):
                put(s, "v", tt)
            for s, tt in zip((96, 97, 98, 99), range(12, 16)):
                put(s, "v", tt)
            # Q/K units: (ot, tch) consumed in phase tch by heads
            # 2*(ot%4), 2*(ot%4)+1 at slot phasestart + h*(2*tch+2)
            put(2, "qk", 2, 0)
            put(4, "qk", 6, 0)
            put(6, "qk", 3, 0)
            put(8, "qk", 7, 0)
            for pos, (s0, s1) in zip(range(4), ((10, 12), (14, 17),
                                                (20, 25), (30, 35))):
                put(s0, "qk", pos, 1)
                put(s1, "qk", 4 + pos, 1)
            for pos, (s0, s1) in zip(range(4), ((41, 42), (50, 51),
                                                (58, 64), (70, 76))):
                put(s0, "qk", pos, 2)
                put(s1, "qk", 4 + pos, 2)
            for pos, (s0, s1) in zip(range(4), ((86, 89), (100, 106),
                                                (118, 126), (134, 142))):
                put(s0, "qk", pos, 3)
                put(s1, "qk", 4 + pos, 3)

            giter = [0]
            oproj_q = []
            # O-proj(J) batches become ready at slots 17/49/97; spread each
            # across the following stretch instead of draining eagerly.
            opop = {}
            for s in range(19, 27):
                opop[s] = 1
            for s in range(55, 87, 4):
                opop[s] = 1
            for s in list(range(105, 154, 8)) + [159]:
                opop[s] = 1

            def pop_fill():
                g = giter[0]
                giter[0] += 1
                for u in sched.get(g, []):
                    if u[0] == "v":
                        emit_v(u[1])
                    else:
                        emit_qk(u[1], u[2])
                for _ in range(opop.get(g, 0)):
                    if oproj_q:
                        emit_oproj(*oproj_q.pop(0))

            # ---- attention: one flat pair stream over (h, J, p), with the
            # P@V matmuls lagging one pair behind the exps. The lag crosses
            # J and head boundaries, so the pipeline never drains until the
            # very end (each stall has the next pair's scores as cover).
            def av(h, J, pt, offs, otr):
                for i, coff, qlo, span in offs:
                    qloc = qlo - J * 512
                    nc.tensor.matmul(
                        otr[:, qloc:512],
                        lhsT=VP[:, i, h, :],
                        rhs=pt[:, coff:coff + span],
                        start=(i == 0), stop=(i == 4 * J + 3),
                    )

            def normalize(h, J, otr):
                # den on rows 0:64 (base 0), O on rows 64:128. The very last
                # one is split per q-tile so the tail O-proj units start as
                # soon as their OT columns are ready.
                prow = (h % 2) * 64
                final = h == HPC - 1 and J == 3
                rd = dnp.tile([128, 512], f32, tag="d", name=f"rd{h}_{J}")
                chunks = [(0, 512)] if not final else [
                    (c * 128, 128) for c in range(4)]
                for c0, cw in chunks:
                    nc.vector.reciprocal_approx_fast(
                        rd[0:64, c0:c0 + cw], otr[0:64, c0:c0 + cw])
                    nc.vector.tensor_tensor(
                        out=OT[prow:prow + 64, h // 2,
                               J * 512 + c0:J * 512 + c0 + cw],
                        in0=otr[64:128, c0:c0 + cw], in1=rd[0:64, c0:c0 + cw],
                        op=Mult,
                    )
                if h == HPC - 1:
                    for tq in range(4 * J, 4 * J + 4):
                        for oc2 in range(D // 512):
                            oproj_q.append((tq, oc2))

            # prologue: what phase J0's first heads need
            emit_qk(0, 0)
            emit_qk(4, 0)
            emit_qk(1, 0)
            emit_qk(5, 0)

            # J-outer stream; at each phase turn the last head's pairs are
            # interleaved with the next phase's first head so the score
            # pipeline refills while the old phase's exps drain.
            phases = [[(h, J, p) for h in range(HPC)
                       for p in range(2 * J + 2)]
                      for J in range(T // 512)]
            stream = []
            for J in range(len(phases)):
                items = phases[J]
                if J + 1 < len(phases):
                    k = 2 * J + 2
                    head_n = 2 * (J + 1) + 2
                    tail = items[-k:]
                    head = phases[J + 1][:head_n]
                    phases[J + 1] = phases[J + 1][head_n:]
                    stream.extend(items[:-k])
                    for i in range(max(k, head_n)):
                        if i < k:
                            stream.append(tail[i])
                        if i < head_n:
                            stream.append(head[i])
                else:
                    stream.extend(items)
            pends = []     # (h, J, pt, offs, otr, is_last_pair)
            otrs = {}      # per-(h, J): interleaved groups overlap at turns
            for h, J, p in stream:
                if p == 0:
                    otrs[(h, J)] = psO.tile([128, 512], f32, tag="o",
                                            name=f"otr{h}_{J}")
                otr = otrs[(h, J)]
                prow = (h % 2) * 64
                QTh = QKT[prow:prow + 64, h // 2, :]
                KTh = QKT[prow:prow + 64, 4 + h // 2, :]
                ps = psS.tile([128, 1024], f32, tag="s", name=f"s{h}_{J}_{p}")
                pt = ptp.tile([128, 1024], bf16, tag="p",
                              name=f"pt{h}_{J}_{p}")
                offs = []
                col = 0
                for i in (2 * p, 2 * p + 1):
                    qlo = max(J * 512, i * 128)
                    span = (J + 1) * 512 - qlo
                    # each matmul region must stay within one bank
                    assert col // 512 == (col + span - 1) // 512
                    nc.tensor.matmul(
                        ps[:, col:col + span],
                        lhsT=KTh[:, i * 128:(i + 1) * 128],
                        rhs=QTh[:, qlo:qlo + span],
                        start=True, stop=True,
                    )
                    offs.append((i, col, qlo, span))
                    col += span
                nc.scalar.activation(
                    out=pt[:, 0:col], in_=ps[:, 0:col], func=Exp, scale=0.125)
                # diagonal tiles: zero the upper triangle after exp. GpSimd
                # keeps DVE off the chain, except phase J0 whose short iters
                # Pool would pace (every J0 tile is diagonal).
                meng = nc.vector if J == 0 else nc.gpsimd
                for i, coff, qlo, span in offs:
                    if i >= 4 * J:
                        meng.tensor_tensor(
                            out=pt[:, coff:coff + 128],
                            in0=pt[:, coff:coff + 128], in1=TRI[:],
                            op=Mult,
                        )
                pop_fill()
                # P@V lags 2-3 pairs behind: hides exp + mask latency. Lag 3
                # is PSUM-safe only for J>=1 (J0 passes are just 2 pairs, so
                # a deeper lag would keep 3 otr tiles alive at once).
                lag = 2 if (J == 0 and h < 7) else 3
                while len(pends) >= lag:
                    pd = pends.pop(0)
                    av(*pd[:5])
                    if pd[5]:
                        normalize(pd[0], pd[1], pd[4])
                pends.append((h, J, pt, offs, otr, p == 2 * J + 1))
            for pd in pends:
                av(*pd[:5])
                if pd[5]:
                    normalize(pd[0], pd[1], pd[4])
            while oproj_q:
                emit_oproj(*oproj_q.pop(0))

            if debug:
                nc.sync.dma_start(
                    out=d_qkt[:, :], in_=QKT[:].rearrange("p a t -> p (a t)"))
                nc.sync.dma_start(
                    out=d_vp[:, :], in_=VP[:].rearrange("p a b c -> p (a b c)"))
                nc.sync.dma_start(
                    out=d_ot[:, :], in_=OT[:].rearrange("p a t -> p (a t)"))

    nc.compile()
    return nc


def _in_maps(x, W_qkv, b_qkv, W_o, b_o):
    x = np.asarray(x, np.float32)
    W_qkv = np.asarray(W_qkv, np.float32)
    b_qkv = np.asarray(b_qkv, np.float32)
    W_o = np.asarray(W_o, np.float32)
    b_o = np.asarray(b_o, np.float32)

    maps = []
    for c in range(N_CORES):
        b, hh = c // 2, c % 2
        rs = slice(hh * OC, (hh + 1) * OC)
        wq = W_qkv[0 * D:1 * D][rs]            # [512, 1024]
        wk = W_qkv[1 * D:2 * D][rs]
        wv = W_qkv[2 * D:3 * D][rs]
        wqkT = np.concatenate([wq, wk], 0).T   # [1024, 1024]
        bqv = b_qkv[0 * D:1 * D][rs]
        bvv = b_qkv[2 * D:3 * D][rs]
        # V bias folded into the output bias: attn rows sum to 1, so
        # out_core = O_nobv @ Wo_core.T + (0.5*b_o + Wo[:, rs] @ bv_core)
        bo_core = 0.5 * b_o + W_o[:, rs] @ bvv
        tri = np.triu(np.ones((128, 128), np.float32))
        maps.append({
            "xT": np.ascontiguousarray(x[b].T).astype(BF16),
            "wqkT": np.ascontiguousarray(wqkT).astype(BF16),
            "wvT": np.ascontiguousarray(wv.T).astype(BF16),
            "woT": np.ascontiguousarray(W_o[:, rs].T).astype(BF16),
            "bq": np.ascontiguousarray(bqv.reshape(OC // 128, 128).T),
            "bo": bo_core.reshape(1, D),
            "tri": tri.astype(BF16),
        })
    return maps


def _run(x, W_qkv, b_qkv, W_o, b_o, trace=False, tmpdir=None):
    from concourse.bass_utils import run_bass_kernel_spmd

    if "nc" not in _cache:
        _cache["nc"] = _build()
    res = run_bass_kernel_spmd(
        _cache["nc"], _in_maps(x, W_qkv, b_qkv, W_o, b_o),
        core_ids=list(range(N_CORES)), trace=trace, tmpdir=tmpdir,
    )
    out = np.empty((B, T, D), np.float32)
    for b in range(B):
        out[b] = (res.results[2 * b]["out"].astype(np.float32)
                  + res.results[2 * b + 1]["out"].astype(np.float32))
    return out, res


def kernel(x, W_qkv, b_qkv, W_o, b_o):
    out, _ = _run(x, W_qkv, b_qkv, W_o, b_o, trace=False)
    return out
